# revision 3
# baseline (speedup 1.0000x reference)
"""Group-equivariant depthwise conv (C4) on 8 Trainium2 NeuronCores.

out[b, r*C+c] = crosscorr(x[b, c], rot90(weight[c, 0], r)), r in 0..3
x: [16, 192, 128, 128] f32, weight: [192, 1, 3, 3] f32 -> out: [16, 768, 128, 128].

Sharding: data-parallel over batch (2 images per core). Per core the
(b, c) rows flatten to 384 partition-rows = 3 chunks of 128. Per chunk,
H is tiled by 32 rows into zero-padded SBUF tiles [128, 34, 130]
(1-pixel halo). All taps are per-channel scalar MACs:
  - rotations 0-2 run on the TensorEngine as 9 PSUM-accumulated matmuls
    with diagonal float32r stationary matrices (diag(w_r[c, tap]));
  - rotation 3 runs on the Vector engine as scalar_tensor_tensor MACs,
    with the center tap initialized by the Scalar engine (activation
    Copy with per-partition scale);
  - the Scalar engine drains PSUM banks to SBUF.
"""

import numpy as np
from contextlib import ExitStack

from concourse import bacc, mybir, tile
from concourse.bass_utils import run_bass_kernel_spmd

B, C, H, W = 16, 192, 128, 128
NCORES = 8
BS = B // NCORES            # batches per core
ROWS = BS * C               # 384 (b,c) rows per core
NCHUNK = ROWS // 128        # 3
HT = 32                     # output rows per h-tile
NHT = H // HT               # 4
SUB = 4                     # output rows per PSUM subtile
NSUB = HT // SUB            # 8
NROT_PE = 3                 # rotations on the TensorEngine; rotation 3 on DVE
TW = W + 2                  # padded tile width

F32 = mybir.dt.float32
F32R = mybir.dt.float32r

# tap order: center first (initializes the PSUM accumulation group)
TAPS = [(1, 1)] + [(ti, tj) for ti in range(3) for tj in range(3) if (ti, tj) != (1, 1)]

# partition segments of each chunk: (p0, n, b_local, c0)
CHUNK_SEGS = []
for _ch in range(NCHUNK):
    segs = []
    g = _ch * 128
    while g < (_ch + 1) * 128:
        b_loc, c0 = g // C, g % C
        n = min((_ch + 1) * 128 - g, C - c0)
        segs.append((g - _ch * 128, n, b_loc, c0))
        g += n
    CHUNK_SEGS.append(segs)


def _build():
    nc = bacc.Bacc("TRN2", target_bir_lowering=False, debug=False, num_devices=NCORES)
    x_d = nc.dram_tensor("x", [ROWS, H, W], F32, kind="ExternalInput").ap()
    w_d = nc.dram_tensor("w36", [ROWS, 36], F32, kind="ExternalInput").ap()
    o_d = nc.dram_tensor("out", [BS * 4 * C, H, W], F32, kind="ExternalOutput").ap()

    with tile.TileContext(nc) as tc, ExitStack() as ctx:
        xpool = ctx.enter_context(tc.tile_pool(name="xraw", bufs=2))
        xrpool = ctx.enter_context(tc.tile_pool(name="xr", bufs=2))
        opool = ctx.enter_context(tc.tile_pool(name="osb", bufs=3))
        wpool = ctx.enter_context(tc.tile_pool(name="wsb", bufs=2))
        dpool = ctx.enter_context(tc.tile_pool(name="diag", bufs=1))
        pspool = ctx.enter_context(tc.tile_pool(name="ps", bufs=8, space="PSUM"))

        for ch in range(NCHUNK):
            g0 = ch * 128
            # per-chunk weights: [128, 36] fp32 (col r*9 + ti*3 + tj)
            w_sb = wpool.tile([128, 36], F32, tag="wsb")
            nc.sync.dma_start(w_sb[:], w_d[g0 : g0 + 128, :])
            # 36 diagonal stationary matrices, built fp32 then rounded to f32r
            diag_f = dpool.tile([128, 36, 128], F32, tag="df")
            nc.gpsimd.affine_select(
                out=diag_f[:],
                in_=w_sb[:].broadcast_to([128, 36, 128]),
                compare_op=mybir.AluOpType.is_equal,
                fill=0.0,
                base=0,
                pattern=[[0, 36], [-1, 128]],
                channel_multiplier=1,
            )
            diag_r = dpool.tile([128, 36, 128], F32R, tag="dr")
            nc.vector.tensor_copy(diag_r[:], diag_f[:])

            for ht in range(NHT):
                h0 = ht * HT
                # x tile rows t_r map to image rows h0 - 1 + t_r; col c_t to w = c_t - 1
                xt = xpool.tile([128, HT + 2, TW], F32, tag="xraw")
                # zero halo columns; zero halo row at image top/bottom
                nc.gpsimd.memset(xt[:, :, 0:1], 0.0)
                nc.gpsimd.memset(xt[:, :, TW - 1 : TW], 0.0)
                if ht == 0:
                    nc.gpsimd.memset(xt[:, 0:1, :], 0.0)
                    nc.sync.dma_start(
                        xt[:, 1 : HT + 2, 1 : W + 1], x_d[g0 : g0 + 128, 0 : HT + 1, :]
                    )
                elif ht == NHT - 1:
                    nc.gpsimd.memset(xt[:, HT + 1 : HT + 2, :], 0.0)
                    nc.sync.dma_start(
                        xt[:, 0 : HT + 1, 1 : W + 1], x_d[g0 : g0 + 128, h0 - 1 : H, :]
                    )
                else:
                    nc.sync.dma_start(
                        xt[:, :, 1 : W + 1], x_d[g0 : g0 + 128, h0 - 1 : h0 + HT + 1, :]
                    )
                xr = xrpool.tile([128, HT + 2, TW], F32R, tag="xr")
                nc.vector.tensor_copy(xr[:], xt[:])
                xr32 = xr[:].bitcast(F32)

                # ---- rotations 0..2 on the TensorEngine ----
                for r in range(NROT_PE):
                    osb = opool.tile([128, HT, W], F32, tag="osb")
                    for s in range(NSUB):
                        ps = pspool.tile([128, SUB, W], F32, tag="ps")
                        for k, (ti, tj) in enumerate(TAPS):
                            nc.tensor.matmul(
                                ps[:],
                                diag_r[:, r * 9 + ti * 3 + tj, :],
                                xr[:, SUB * s + ti : SUB * s + ti + SUB, tj : tj + W],
                                start=(k == 0),
                                stop=(k == len(TAPS) - 1),
                            )
                        nc.scalar.activation(
                            osb[:, SUB * s : SUB * s + SUB, :],
                            ps[:],
                            mybir.ActivationFunctionType.Copy,
                        )
                    for p0, n, b_loc, c0 in CHUNK_SEGS[ch]:
                        row0 = b_loc * 4 * C + r * C + c0
                        nc.sync.dma_start(
                            o_d[row0 : row0 + n, h0 : h0 + HT, :], osb[p0 : p0 + n, :, :]
                        )

                # ---- rotation 3: ACT center-tap init + DVE accumulate ----
                r = 3
                osb = opool.tile([128, HT, W], F32, tag="osb")
                nc.scalar.activation(
                    osb[:],
                    xr32[:, 1 : HT + 1, 1 : W + 1],
                    mybir.ActivationFunctionType.Copy,
                    scale=w_sb[:, r * 9 + 4 : r * 9 + 5],
                )
                for ti, tj in TAPS[1:]:
                    nc.vector.scalar_tensor_tensor(
                        out=osb[:],
                        in0=xr32[:, ti : ti + HT, tj : tj + W],
                        scalar=w_sb[:, r * 9 + ti * 3 + tj : r * 9 + ti * 3 + tj + 1],
                        in1=osb[:],
                        op0=mybir.AluOpType.mult,
                        op1=mybir.AluOpType.add,
                    )
                for p0, n, b_loc, c0 in CHUNK_SEGS[ch]:
                    row0 = b_loc * 4 * C + r * C + c0
                    nc.sync.dma_start(
                        o_d[row0 : row0 + n, h0 : h0 + HT, :], osb[p0 : p0 + n, :, :]
                    )

    nc.compile()
    return nc


_NC = None


def _get_nc():
    global _NC
    if _NC is None:
        _NC = _build()
    return _NC


def _make_w36(weight):
    w36 = np.zeros((C, 36), dtype=np.float32)
    base = weight[:, 0]  # [C, 3, 3]
    for r in range(4):
        wr = np.rot90(base, r, axes=(1, 2))
        w36[:, r * 9 : (r + 1) * 9] = wr.reshape(C, 9)
    return np.tile(w36, (BS, 1))  # [ROWS, 36]


def kernel(x, weight):
    x = np.asarray(x, dtype=np.float32)
    weight = np.asarray(weight, dtype=np.float32)
    w36 = _make_w36(weight)
    in_maps = [
        {"x": np.ascontiguousarray(x[BS * k : BS * (k + 1)].reshape(ROWS, H, W)), "w36": w36}
        for k in range(NCORES)
    ]
    nc = _get_nc()
    res = run_bass_kernel_spmd(nc, in_maps, list(range(NCORES))).results
    out = np.empty((B, 4 * C, H, W), dtype=np.float32)
    for k in range(NCORES):
        out[BS * k : BS * (k + 1)] = res[k]["out"].reshape(BS, 4 * C, H, W)
    return out
